# revision 4
# baseline (speedup 1.0000x reference)
"""Trainium2 Bass kernel for nn_FAquantizer (4x cascaded residual VQ).

Sharding: pure data parallel - core b handles batch b (B=8 across 8 cores).

Algorithm per core (batch), per T-tile of 512 frames:
  - U = W_all @ x  (all 8 stages' down-projections of the raw input, one
    K=1024 matmul chain into PSUM; per-stage input-proj bias added on the
    PSUM->SBUF copy)
  - per stage s: z_e_s = U_s - sum_j (W_s @ OW_j) z_q_j  (the residual chain
    collapsed into D-space via host-precomputed 8x8 correction matrices; one
    K=64 matmul over the accumulated codes + identity matmul over U_s)
  - dist = cbn_s^T @ z_e_s in [token, code] layout (argmax of the dot with a
    normalized codebook is scale-invariant, so z_e is left unnormalized)
  - DVE max/max_index over 1024 codes -> idx; indirect-DMA gather of code
    vectors; PE transpose back to [d, t]
  - outputs z_p/z_c/z_t/z_r/outs as grouped up-projection matmuls from the
    accumulated codes; out-proj biases (and the per-batch residual mask,
    baked into the outs weights host-side) added on the PSUM->SBUF copies
  - commit/cbl from sum((z_e - z_q)^2) accumulated on device, finished on host
"""
import numpy as np

B, C, T = 8, 1024, 4096
K, D = 1024, 8
NS = 8                      # total VQ stages: p0, c0, c1, t0, t1, r0, r1, r2
TT = 512                    # T-tile
NTILES = T // TT
NCHUNK = C // 128           # 8 c-chunks
TCH = TT // 128             # 4 token chunks per tile

STAGES = [("p", 0), ("c", 0), ("c", 1), ("t", 0), ("t", 1), ("r", 0), ("r", 1), ("r", 2)]
SUBSETS = [[], [], [1], [0, 1, 2], [0, 1, 2, 3], [0, 1, 2, 3, 4],
           [0, 1, 2, 3, 4, 5], [0, 1, 2, 3, 4, 5, 6]]
GROUPS = {"z_p": (0, 8), "z_c": (8, 24), "z_t": (24, 40), "z_r": (40, 64), "outs": (0, 64)}
OUTNAMES = ["outs", "z_p", "z_c", "z_t", "z_r"]

_COMPILED = {}


def _legalize_multiwait(nc):
    """walrus here encodes at most 1 sync-wait per instruction (2 for
    EventSemaphore); move excess waits onto inserted same-engine
    EventSemaphores placed immediately before the capped instruction."""
    import bass_rust as br
    import concourse.mybir as mybir
    n_fixed = 0
    fid = [0]
    for fn in nc.m.functions:
        for bb in fn.blocks:
            insts = bb.instructions
            out = []
            changed = False
            for inst in insts:
                si = inst.sync_info
                if si is None:
                    out.append(inst)
                    continue
                waits = list(si.on_wait)
                cap = 2 if isinstance(inst, mybir.InstEventSemaphore) else 1
                if len(waits) > cap:
                    extra, keep = waits[:-cap], waits[-cap:]
                    while extra:
                        chunk, extra = extra[:2], extra[2:]
                        fid[0] += 1
                        es = br.InstEventSemaphore(name=f"mwfix_{fid[0]}")
                        es.engine = inst.engine
                        es.sync_info = br.SyncInfo(on_wait=chunk, on_update=[])
                        out.append(es)
                    inst.sync_info = br.SyncInfo(on_wait=keep, on_update=list(si.on_update))
                    n_fixed += 1
                    changed = True
                out.append(inst)
            if changed:
                il = bb.instructions
                il.clear()
                il.extend(out)
    return n_fixed


def _build_consts(inputs):
    """Host-side constant precompute in float64, cast to fp32."""
    W, bvec, CB, CBN, OW, OB = [], [], [], [], [], []
    for nm, i in STAGES:
        W.append(inputs[f"{nm}_in_w"][i].astype(np.float64))
        bvec.append(inputs[f"{nm}_in_b"][i].astype(np.float64))
        cb = inputs[f"{nm}_cb"][i].astype(np.float64)
        CB.append(cb)
        n = np.sqrt((cb * cb).sum(-1, keepdims=True))
        CBN.append(cb / np.maximum(n, 1e-12))
        OW.append(inputs[f"{nm}_out_w"][i].astype(np.float64))
        OB.append(inputs[f"{nm}_out_b"][i].astype(np.float64))

    f32 = np.float32
    Wst = np.concatenate(W, 0)                        # [64, C]
    UW = np.zeros((128, NCHUNK, 64), f32)             # lhsT chunks for U
    for k in range(NCHUNK):
        UW[:, k, :] = Wst.T[128 * k:128 * k + 128, :].astype(f32)

    # corr lhsT [128, NS, 8]: rows 0-63 = -M coefficients over accumulated codes,
    # rows 64-127 = one-hot selector picking U_s out of the U block of STATE
    corrW = np.zeros((128, NS, 8), f32)
    zebias = np.zeros((8, NS), np.float64)
    for s in range(NS):
        for d in range(8):
            corrW[64 + 8 * s + d, s, d] = 1.0
        for j in SUBSETS[s]:
            M = W[s] @ OW[j]                          # [8, 8] d_out x d_in
            corrW[8 * j:8 * j + 8, s, :] = (-M.T).astype(f32)
            zebias[:, s] -= W[s] @ OB[j]
    # U-copy bias: per U row 8s+d: input-proj bias + correction constant
    ubias = np.zeros((64, 1), f32)
    for s in range(NS):
        for d in range(8):
            ubias[8 * s + d, 0] = f32(bvec[s][d] + zebias[d, s])

    cbnT = np.zeros((8, NS, K), f32)                  # [d, stage, code]
    for s in range(NS):
        cbnT[:, s, :] = CBN[s].T.astype(f32)
    cb_rows = np.concatenate([c.astype(f32) for c in CB], 0)    # [8192, 8]

    def owstack(sts, scale=1.0):
        # zero-padded to the full 64 code rows so the matmul rhs is STATE[0:64]
        w = np.zeros((64, NCHUNK, 128), f32)
        for s in sts:
            for k in range(NCHUNK):
                w[8 * s:8 * s + 8, k, :] = (scale * OW[s].T[:, 128 * k:128 * k + 128]).astype(f32)
        return w

    ows = {"z_p": owstack([0]), "z_c": owstack([1, 2]), "z_t": owstack([3, 4]),
           "z_r": owstack([5, 6, 7])}
    obs = {"z_p": OB[0], "z_c": OB[1] + OB[2], "z_t": OB[3] + OB[4],
           "z_r": OB[5] + OB[6] + OB[7]}

    # per-batch residual mask (host-computed from flags)
    noise = inputs["noise_added_flags"]
    recon = inputs["recon_noisy_flags"]
    rr = inputs["res_rand"].astype(np.float64)
    mask = np.where(noise & recon, 1.0, np.where(noise & ~recon, 0.0, rr))  # [B]

    per_core = []
    for b in range(B):
        owo = ows["z_p"] + ows["z_c"] + ows["z_t"] + owstack([5, 6, 7], mask[b])
        ob_outs = obs["z_p"] + obs["z_c"] + obs["z_t"] + mask[b] * obs["z_r"]
        obias = np.zeros((128, 5, NCHUNK), f32)
        for t_i, nm in enumerate(OUTNAMES):
            src = ob_outs if nm == "outs" else obs[nm]
            for k in range(NCHUNK):
                obias[:, t_i, k] = src[128 * k:128 * k + 128].astype(f32)
        per_core.append({"owo": owo, "obias": obias})

    shared = {
        "UW": UW, "ubias": ubias, "corrW": corrW, "cbnT": cbnT,
        "cb_rows": cb_rows, "owp": ows["z_p"], "owc": ows["z_c"],
        "owt": ows["z_t"], "owr": ows["z_r"],
        "eye128": np.eye(128, dtype=f32),
    }
    return shared, per_core


def _build_bass():
    import concourse.bass as bass
    import concourse.tile as tile
    import concourse.mybir as mybir
    from contextlib import ExitStack

    f32 = mybir.dt.float32
    nc = bass.Bass()
    dp = nc.declare_dram_parameter
    x_in = dp("x", [C, T], f32, isOutput=False)
    UW_in = dp("UW", [128, NCHUNK, 64], f32, isOutput=False)
    ubias_in = dp("ubias", [64, 1], f32, isOutput=False)
    corrW_in = dp("corrW", [128, NS, 8], f32, isOutput=False)
    cbnT_in = dp("cbnT", [8, NS, K], f32, isOutput=False)
    cb_rows_in = dp("cb_rows", [NS * K, 8], f32, isOutput=False)
    ow_in = {nm: dp(f"ow{nm[-1]}" if nm != "outs" else "owo", [64, NCHUNK, 128], f32,
                    isOutput=False) for nm in OUTNAMES}
    obias_in = dp("obias", [128, 5, NCHUNK], f32, isOutput=False)
    eye128_in = dp("eye128", [128, 128], f32, isOutput=False)

    z_out = {nm: dp(nm, [C, T], f32, isOutput=True) for nm in OUTNAMES}
    loss_out = dp("loss", [8, 1], f32, isOutput=True)

    with tile.TileContext(nc) as tc, ExitStack() as ctx:
        consts = ctx.enter_context(tc.tile_pool(name="consts", bufs=1))
        xin = ctx.enter_context(tc.tile_pool(name="xin", bufs=2))
        state = ctx.enter_context(tc.tile_pool(name="state", bufs=1))
        small = ctx.enter_context(tc.tile_pool(name="small", bufs=3))
        ostage = ctx.enter_context(tc.tile_pool(name="ostage", bufs=3))
        dist_pp = ctx.enter_context(tc.tile_pool(name="dpp", bufs=2, space="PSUM"))
        ze_pp = ctx.enter_context(tc.tile_pool(name="zepp", bufs=2, space="PSUM"))
        u_pp = ctx.enter_context(tc.tile_pool(name="upp", bufs=1, space="PSUM"))
        o_pp = ctx.enter_context(tc.tile_pool(name="opp", bufs=1, space="PSUM"))

        def load_const(ap, shape, name):
            t = consts.tile(shape, f32, name=name)
            nc.sync.dma_start(out=t[:], in_=ap)
            return t

        UW_sb = load_const(UW_in[:], [128, NCHUNK, 64], "uw")
        ubias_sb = load_const(ubias_in[:], [64, 1], "ubiasc")
        corrW_sb = load_const(corrW_in[:], [128, NS, 8], "corrw")
        cbnT_sb = load_const(cbnT_in[:], [8, NS, K], "cbnt")
        ow_sb = {nm: load_const(ow_in[nm][:], [64, NCHUNK, 128], f"ow_{nm}")
                 for nm in OUTNAMES}
        obias_sb = load_const(obias_in[:], [128, 5, NCHUNK], "obias")
        eye128_sb = load_const(eye128_in[:], [128, 128], "eye128c")

        x_dram = x_in.rearrange("(c p) t -> p c t", p=128)
        zdram = {nm: z_out[nm].rearrange("(c p) t -> p c t", p=128) for nm in OUTNAMES}

        STATE = state.tile([128, TT], f32)       # rows 0-63 codes, 64-127 U
        ze_all = state.tile([8, NS, TT], f32)
        zq_tile = state.tile([8, NS, TT], f32)
        diff_sb = state.tile([8, NS, TT], f32)
        loss_acc = state.tile([8, 1], f32)
        lp_sb = state.tile([8, 1], f32)


        nc.vector.memset(STATE[:], 0.0)
        nc.vector.memset(loss_acc[:], 0.0)

        ID = mybir.ActivationFunctionType.Identity
        for it in range(NTILES):
            tsl = slice(it * TT, (it + 1) * TT)
            x_tile = xin.tile([128, NCHUNK, TT], f32)
            nc.sync.dma_start(out=x_tile[:], in_=x_dram[:, :, tsl])

            # U = W_all @ x (+ bias on copy)
            u_ps = u_pp.tile([64, TT], f32)
            for kc in range(NCHUNK):
                nc.tensor.matmul(out=u_ps[:], lhsT=UW_sb[:, kc, :], rhs=x_tile[:, kc, :],
                                 start=(kc == 0), stop=(kc == NCHUNK - 1))
            nc.scalar.activation(out=STATE[64:128, :], in_=u_ps[:], func=ID,
                                 bias=ubias_sb[:], scale=1.0)

            for s in range(NS):
                # z_e_s = U_s - corrections (one K=128 matmul over STATE)
                ze_ps = ze_pp.tile([8, TT], f32, tag="ze8", name="ze_ps")
                nc.tensor.matmul(out=ze_ps[:], lhsT=corrW_sb[:, s, :], rhs=STATE[:],
                                 start=True, stop=True)
                nc.scalar.copy(out=ze_all[:, s, :], in_=ze_ps[:])

                zqt_ps = ze_pp.tile([8, TT], f32, tag="ze8", name="zqt_ps")
                for tcn in range(TCH):
                    csl = slice(tcn * 128, (tcn + 1) * 128)
                    dist_ps = dist_pp.tile([128, K], f32, tag="dist", name="dist_ps")
                    nc.tensor.matmul(out=dist_ps[:, 0:512],
                                     lhsT=ze_all[:, s, csl],
                                     rhs=cbnT_sb[:, s, 0:512], start=True, stop=True)
                    nc.tensor.matmul(out=dist_ps[:, 512:1024],
                                     lhsT=ze_all[:, s, csl],
                                     rhs=cbnT_sb[:, s, 512:1024], start=True, stop=True)
                    mx8 = small.tile([128, 8], f32, tag="mx8", name="mx8")
                    idx8 = small.tile([128, 8], mybir.dt.uint32, tag="idx8", name="idx8")
                    nc.vector.max(out=mx8[:], in_=dist_ps[:])
                    nc.vector.max_index(out=idx8[:], in_max=mx8[:], in_values=dist_ps[:])
                    idxo = small.tile([128, 1], mybir.dt.int32, tag="idxo", name="idxo")
                    nc.vector.tensor_scalar(out=idxo[:], in0=idx8[:, 0:1],
                                            scalar1=s * K, scalar2=None,
                                            op0=mybir.AluOpType.add)
                    zq_td = small.tile([128, 8], f32, tag="zqtd", name="zq_td")
                    nc.gpsimd.indirect_dma_start(
                        out=zq_td[:], out_offset=None, in_=cb_rows_in[:],
                        in_offset=bass.IndirectOffsetOnAxis(ap=idxo[:, 0:1], axis=0))
                    nc.tensor.transpose(out=zqt_ps[:, csl], in_=zq_td[:], identity=eye128_sb[:])
                nc.scalar.copy(out=zq_tile[:, s, :], in_=zqt_ps[:])
                nc.sync.dma_start(out=STATE[8 * s:8 * s + 8, :], in_=zq_tile[:, s, :])

            # loss accumulation: sum((z_e - z_q)^2)
            nc.vector.tensor_tensor(out=diff_sb[:], in0=ze_all[:], in1=zq_tile[:],
                                    op=mybir.AluOpType.subtract)
            nc.scalar.activation(out=diff_sb[:], in_=diff_sb[:],
                                 func=mybir.ActivationFunctionType.Square,
                                 accum_out=lp_sb[:])
            nc.vector.tensor_tensor(out=loss_acc[:], in0=loss_acc[:], in1=lp_sb[:],
                                    op=mybir.AluOpType.add)

            # grouped output up-projections (zero-padded weights, rhs = code rows)
            for t_i, nm in enumerate(OUTNAMES):
                ost = ostage.tile([128, NCHUNK, TT], f32, tag="ost", name=f"ost_{nm}{it}")
                for kc in range(NCHUNK):
                    o_ps = o_pp.tile([128, TT], f32, tag="ops", name="o_ps")
                    nc.tensor.matmul(out=o_ps[:], lhsT=ow_sb[nm][:, kc, :],
                                     rhs=STATE[0:64, :], start=True, stop=True)
                    nc.scalar.activation(out=ost[:, kc, :], in_=o_ps[:], func=ID,
                                         bias=obias_sb[:, t_i, kc:kc + 1], scale=1.0)
                nc.sync.dma_start(out=zdram[nm][:, :, tsl], in_=ost[:])

        nc.sync.dma_start(out=loss_out[:], in_=loss_acc[:])

    _legalize_multiwait(nc)
    return nc


def kernel(**inputs):
    from concourse.bass_utils import run_bass_kernel_spmd

    if "nc" not in _COMPILED:
        _COMPILED["nc"] = _build_bass()
    nc = _COMPILED["nc"]

    shared, per_core = _build_consts(inputs)
    x = np.ascontiguousarray(inputs["x"], dtype=np.float32)
    in_maps = []
    for b in range(B):
        m = dict(shared)
        m["x"] = x[b]
        m["owo"] = per_core[b]["owo"]
        m["obias"] = per_core[b]["obias"]
        in_maps.append(m)

    res = run_bass_kernel_spmd(nc, in_maps, core_ids=list(range(B)))
    outs = np.stack([res.results[b]["outs"] for b in range(B)])
    z_p = np.stack([res.results[b]["z_p"] for b in range(B)])
    z_c = np.stack([res.results[b]["z_c"] for b in range(B)])
    z_t = np.stack([res.results[b]["z_t"] for b in range(B)])
    z_r = np.stack([res.results[b]["z_r"] for b in range(B)])
    loss = np.stack([res.results[b]["loss"] for b in range(B)])  # [B, 64, 1]
    commit = np.float32(loss.astype(np.float64).sum() / (D * T) / B)
    cbl = commit
    return (outs, z_p, z_c, z_t, z_r, commit, cbl)
